# revision 58
# baseline (speedup 1.0000x reference)
"""Multi-head causal attention on 8 Trainium2 NeuronCores.

Sharding: core c handles batch b=c//4, head group g=c%4 (4 heads of 16).
Per-core Bass kernel computes QKV projection, causal flash-style attention
(transposed-scores layout), and the out-projection partial; the host sums
the 4 per-batch partials (the out_proj all-reduce) and adds the bias.

Layout notes (per core, S=2048 tokens, D=1024, 4 heads x dh=64):
  - xT [D, S] bf16 arrives pre-transposed from host (d_in on partitions).
  - qT/kT [128, pair, S]: partitions = head-dim; pair p holds heads 2p
    (partitions 0:64) and 2p+1 (64:128); k^T zero-padded per head
    (ktpA=[kA|0], ktpB=[0|kB]) so score matmuls are full-array K=128.
  - scoresT tile [128 k-tokens, 512 q-tokens]; exp on the scalar engine
    (the only ACT table ever loaded), exp row-sums accumulate on DVE,
    diagonal causal masks as a single GpSimd multiply per k-chunk.
  - v zero-padded per head parity so ctx matmuls are full-array (M=128)
    and the two heads accumulate additively in one PSUM bank.
  - PE pipelining: ctx matmuls for k-chunk i are emitted after the score
    matmuls of chunk i+1, so the PE never waits on the exp; independent
    "filler" matmul work (pair-1 QKV projections, deferred out-proj
    blocks) is interleaved into the attention chunks so the PE stays
    busy while scalar/vector tails drain — a mostly-idle PE window
    re-throttles the PE HAM clock gate from 2.4 to 1.2 GHz.
  - Softmax denominators: one col-tiled ones-matmul pair per chunk, DVE
    reciprocal_approx_fast, then a fused psum->sbuf normalize multiply;
    each chunk's finish is emitted after the NEXT chunk's body.
  - out^T partial [D, S] f32 is DMA'd straight from PSUM (no staging).
"""

import sys

sys.path.insert(0, "/opt/trn_rl_repo")

import numpy as np
import ml_dtypes

import concourse.bass as bass
import concourse.tile as tile
from concourse import bacc, mybir
from concourse import bass_utils

BF16 = ml_dtypes.bfloat16
F32 = mybir.dt.float32
BF = mybir.dt.bfloat16

N_CORES = 8
S = 2048          # tokens
D = 1024          # model dim
DHC = 256         # head dims per core (4 heads x 64)
DH = 64
NQ = 4            # q chunks of 512
NK = 16           # k chunks of 128
NO = 8            # d_in / d_out chunks of 128

_NC_CACHE = None


def _build_core_kernel():
    nc = bacc.Bacc("TRN2", target_bir_lowering=False, debug=False,
                   num_devices=N_CORES)
    xT = nc.dram_tensor("xT", [D, S], BF, kind="ExternalInput").ap()
    w_all = nc.dram_tensor("w_all", [D, 3 * DHC], BF, kind="ExternalInput").ap()
    wo = nc.dram_tensor("wo", [DHC, D], BF, kind="ExternalInput").ap()
    masks = nc.dram_tensor("masks", [128, 256], BF, kind="ExternalInput").ap()
    outT = nc.dram_tensor("outT", [D, S], BF, kind="ExternalOutput").ap()

    with tile.TileContext(nc) as tc:
        _emit(tc, xT, w_all, wo, masks, outT)
    nc.compile()
    return nc


def _emit(tc, xT, w_all, wo, masks, outT):
    nc = tc.nc
    EXPF = mybir.ActivationFunctionType.Exp

    from contextlib import ExitStack
    ctx = ExitStack()
    const = ctx.enter_context(tc.tile_pool(name="const", bufs=1))
    work = ctx.enter_context(tc.tile_pool(name="work", bufs=6))
    outp = ctx.enter_context(tc.tile_pool(name="outp", bufs=6))
    nrm = ctx.enter_context(tc.tile_pool(name="nrm", bufs=2))
    ps_mm = ctx.enter_context(tc.tile_pool(name="ps_mm", bufs=2, space="PSUM"))
    ps_s = ctx.enter_context(tc.tile_pool(name="ps_s", bufs=2, space="PSUM"))
    ps_c = ctx.enter_context(tc.tile_pool(name="ps_c", bufs=2, space="PSUM"))

    # ---- persistent SBUF tensors ----
    xt = const.tile([128, NO, S], BF, tag="xt")          # x^T, d_in chunks
    wa = const.tile([128, NO, 3 * DHC], BF, tag="wa")    # [Wq|Wk|Wv] slices
    wos = const.tile([128, 2, D], BF, tag="wos")         # Wo row chunks
    msk = const.tile([128, 2, 128], BF, tag="msk")       # causal staircase x2
    qt = const.tile([128, 2, S], BF, tag="qt")           # q^T per pair
    ktpA = const.tile([128, 2, S], BF, tag="ktpA")
    ktpB = const.tile([128, 2, S], BF, tag="ktpB")
    vsb = const.tile([128, NK, 4 * 128], BF, tag="vsb")
    ctxT = const.tile([128, 2, S], BF, tag="ctxT")       # normalized ctx^T
    ones = const.tile([128, DH], BF, tag="ones")

    # per-chunk DMAs, wa/xt interleaved, so compute starts on chunk 0
    # without waiting for the full weight load
    wao = w_all.rearrange("(o p) f -> p o f", p=128)
    xTo = xT.rearrange("(o p) s -> o p s", p=128)
    for o in range(NO):
        nc.sync.dma_start(wa[:, o, :], wao[:, o, :])
        nc.sync.dma_start(xt[:, o, :], xTo[o])
    nc.sync.dma_start(msk[:], masks.rearrange("p (g f) -> p g f", g=2))
    nc.sync.dma_start(wos[:], wo.rearrange("(c p) f -> p c f", p=128))
    nc.vector.memset(ones[:], 1.0)
    nc.vector.memset(ktpA[:], 0.0)
    nc.gpsimd.memset(ktpB[:], 0.0)
    nc.gpsimd.memset(vsb[:], 0.0)
    msk2 = msk[:]  # both heads' triangular mask in one [128, 2, 128] op

    # ---- QKV projections ----
    def emit_qk02_stream():
        # q and k for pair 0 together, o-loop outermost: 8 matmuls per
        # arriving x^T/w chunk keep the PE saturated (and the HAM clock
        # busy) while the input DMA streams in.  q accumulates in the two
        # ps_s tiles; k's four 512-col accumulators borrow ps_mm + ps_c.
        pq0 = ps_s.tile([128, 1024], F32, tag="ps")
        pq1 = ps_s.tile([128, 1024], F32, tag="ps")
        pk0 = ps_mm.tile([128, 512], F32, tag="mm")
        pk1 = ps_c.tile([128, 512], F32, tag="pc")
        pk2 = ps_mm.tile([128, 512], F32, tag="mm")
        pk3 = ps_c.tile([128, 512], F32, tag="pc")
        pks = [pk0, pk1, pk2, pk3]
        for o in range(NO):
            for n in range(NQ):
                # k before q: k's accumulators gate emit_v (via ps_mm), so
                # their final stop — and psum-freeing copies — land earlier
                n_sl = slice(512 * n, 512 * n + 512)
                nc.tensor.matmul(
                    pks[n][:], lhsT=wa[:, o, 256:384], rhs=xt[:, o, n_sl],
                    start=(o == 0), stop=(o == NO - 1),
                    skip_group_check=True)
                nc.tensor.matmul(
                    [pq0, pq1][n // 2][:, 512 * (n % 2):512 * (n % 2) + 512],
                    lhsT=wa[:, o, 0:128], rhs=xt[:, o, n_sl],
                    start=(o == 0), stop=(o == NO - 1),
                    skip_group_check=True)
        for n in range(NQ):
            n_sl = slice(512 * n, 512 * n + 512)
            nc.vector.tensor_copy(ktpA[0:64, 0, n_sl], pks[n][0:64, :])
            nc.scalar.copy(ktpB[64:128, 0, n_sl], pks[n][64:128, :])

        def qt_copies():  # deferred past emit_v: ps_s isn't needed until
            for g in range(2):  # attention, so don't stall the PE here
                g_sl = slice(1024 * g, 1024 * g + 1024)
                nc.vector.tensor_copy(qt[:, 0, g_sl], [pq0, pq1][g][:])
        return qt_copies

    def qk_fillers(m, ns=tuple(range(NQ)), split_copy=False):
        # pair-1 projections as independent filler units (one 512-token
        # block each: 8 accumulating matmuls + a psum->sbuf copy on the
        # scalar engine, which has slack inside attention windows);
        # split_copy spreads the two k copies across vector+scalar for
        # units that ride inside an exp-paced window
        pp = m % 2

        def make(n):
            def fill():
                n_sl = slice(512 * n, 512 * n + 512)
                pq = ps_mm.tile([128, 512], F32, tag="mm")
                for o in range(NO):
                    nc.tensor.matmul(
                        pq[:], lhsT=wa[:, o, 128 * m:128 * m + 128],
                        rhs=xt[:, o, n_sl],
                        start=(o == 0), stop=(o == NO - 1))
                if m < 2:
                    nc.scalar.copy(qt[:, pp, n_sl], pq[:])
                elif split_copy:
                    nc.vector.tensor_copy(ktpA[0:64, pp, n_sl], pq[0:64, :])
                    nc.scalar.copy(ktpB[64:128, pp, n_sl], pq[64:128, :])
                else:
                    nc.scalar.copy(ktpA[0:64, pp, n_sl], pq[0:64, :])
                    nc.scalar.copy(ktpB[64:128, pp, n_sl], pq[64:128, :])
            return fill
        return [make(n) for n in ns]

    def emit_v(mid=None):
        # v [tokens, 4*dh] = x @ Wv  (x^T chunks are the stationary side)
        for t in range(NK):
            if t == 2 and mid is not None:
                mid()  # deferred copies drain while the PE streams v
            pv = ps_mm.tile([128, 512], F32, tag="mm")
            for o in range(NO):
                nc.tensor.matmul(
                    pv[:, :DHC], lhsT=xt[:, o, 128 * t:128 * t + 128],
                    rhs=wa[:, o, 2 * DHC:3 * DHC],
                    start=(o == 0), stop=(o == NO - 1))
            pv4 = pv[:, :DHC].rearrange("p (h c) -> p h c", c=DH)
            dst4 = vsb.rearrange("p t (h c) -> p t h c", c=128)
            # even head slots hold [v|0], odd hold [0|v].  All copies on
            # DVE: queueing half on the scalar FIFO delays the first
            # attention exps behind ~7us of copies (strict FIFO order)
            nc.vector.tensor_copy(dst4[:, t, 0::2, 0:64], pv4[:, 0::2, :])
            nc.vector.tensor_copy(dst4[:, t, 1::2, 64:128], pv4[:, 1::2, :])

    # ---- attention for one (pair, q-chunk); returns deferred finisher ----
    def emit_attn_body(p, j, fillers=(), fill_at=None):
        h0 = 2 * p
        n_i = 4 * j + 4
        q_sl = slice(512 * j, 512 * j + 512)
        pc = ps_c.tile([128, 512], F32, tag="pc")
        rs = work.tile([128, 1024], BF, tag="rs")  # per-head exp row-sums
        fillers = list(fillers)
        if fill_at is None:
            spacing = max(1, n_i // (len(fillers) + 1)) if fillers else 0
            fill_at = [(k + 1) * spacing - 1 for k in range(len(fillers))]
        nfill = 0

        def emit_ctx(eT, q0, i):
            for h in (0, 1):  # head within pair; full-array M=128 matmuls
                sl = slice(512 * h + q0, 512 * h + 512)
                hl = h0 + h
                nc.tensor.matmul(
                    pc[:, q0:512],
                    lhsT=vsb[:, i, 128 * hl:128 * hl + 128],
                    rhs=eT[:, sl],
                    start=(i == 0 and h == 0), stop=(i == n_i - 1 and h == 1),
                    skip_group_check=True)

        pend = []
        for i in range(n_i):
            k_sl = slice(128 * i, 128 * i + 128)
            d = i - 4 * j
            # diagonal tiles: k-chunk i only reaches q >= 128*d in this
            # q-window; restrict all work to the valid column range.
            q0 = 128 * d if d > 0 else 0
            qv_sl = slice(512 * j + q0, 512 * j + 512)
            pss = ps_s.tile([128, 1024], F32, tag="ps")
            nc.tensor.matmul(pss[:, q0:512],
                             lhsT=ktpA[:, p, k_sl], rhs=qt[:, p, qv_sl],
                             start=True, stop=True)
            nc.tensor.matmul(pss[:, 512 + q0:1024],
                             lhsT=ktpB[:, p, k_sl], rhs=qt[:, p, qv_sl],
                             start=True, stop=True)
            # ctx matmuls run two iterations behind their scores so the PE
            # never waits on the exp -> mask chain of the same k-chunk
            if len(pend) == 2:
                emit_ctx(*pend.pop(0))
            if fillers and nfill < len(fillers) and i >= fill_at[nfill]:
                fillers[nfill]()
                nfill += 1
            eT = work.tile([128, 1024], BF, tag="exp")
            if q0:
                ev = eT.rearrange("p (g f) -> p g f", g=2)[:, :, q0:512]
                pv_ = pss.rearrange("p (g f) -> p g f", g=2)[:, :, q0:512]
                nc.scalar.activation(ev, pv_, EXPF, scale=0.125)
            else:
                nc.scalar.activation(eT[:], pss[:], EXPF, scale=0.125)
            if d >= 0:  # both heads' triangular diagonal mask in one op
                e2 = eT.rearrange("p (g f) -> p g f", g=2)[:, :, q0:q0 + 128]
                nc.vector.tensor_mul(e2, e2, msk2)
            # exp row-sum accumulation (both heads, one DVE op per k-chunk)
            if i == 0:
                nc.vector.tensor_copy(rs[:], eT[:])
            elif q0:
                rv = rs.rearrange("p (g f) -> p g f", g=2)[:, :, q0:512]
                ev2 = eT.rearrange("p (g f) -> p g f", g=2)[:, :, q0:512]
                nc.vector.tensor_add(rv, rv, ev2)
            else:
                nc.vector.tensor_add(rs[:], rs[:], eT[:])
            pend.append((eT, q0, i))
        for pe in pend:
            emit_ctx(*pe)
        for k in range(nfill, len(fillers)):
            fillers[k]()

        def finish():
            # denominators: one col-tiled ones-matmul pair on the summed
            # rows, then 1/den on DVE fused into the psum->sbuf normalize
            pd = ps_mm.tile([128, 512], F32, tag="mm")
            nc.tensor.matmul(pd[0:64, :], lhsT=ones[:], rhs=rs[:, 0:512],
                             start=True, stop=True)
            nc.tensor.matmul(pd[64:128, :], lhsT=ones[:], rhs=rs[:, 512:1024],
                             start=True, stop=True)
            rec = nrm.tile([128, 512], F32, tag="rec")
            nc.vector.reciprocal_approx_fast(rec[:], pd[:])
            nc.vector.tensor_mul(ctxT[:, p, q_sl], pc[:], rec[:])
        return finish

    # ---- out projection: outT[:, n] += wo.T @ ctxT ----
    # bf16 staging, two 128-row blocks per DMA to halve sync-queue issues
    outT_p = outT.rearrange("(mm p) s -> p mm s", p=128)

    def outproj_fillers(n, tail=False):
        n_sl = slice(512 * n, 512 * n + 512)

        def make(m):  # one filler = out rows 128m .. 128(m+2)
            def fill():
                osb = outp.tile([128, 2, 512], BF, tag="osb")
                if tail:
                    # attention psum is free at the tail: use a 2-bank tile
                    # and drain both halves with parallel scalar+DVE copies
                    po2 = ps_s.tile([128, 1024], F32, tag="ps")
                    for k in (0, 1):
                        for p in (0, 1):
                            nc.tensor.matmul(
                                po2[:, 512 * k:512 * k + 512],
                                lhsT=wos[:, p, 128 * (m + k):128 * (m + k) + 128],
                                rhs=ctxT[:, p, n_sl],
                                start=(p == 0), stop=(p == 1),
                                skip_group_check=True)
                    nc.scalar.copy(osb[:, 0, :], po2[:, 0:512])
                    nc.vector.tensor_copy(osb[:, 1, :], po2[:, 512:1024])
                else:
                    for k in (0, 1):
                        po = ps_mm.tile([128, 512], F32, tag="mm")
                        for p in (0, 1):
                            nc.tensor.matmul(
                                po[:], lhsT=wos[:, p, 128 * (m + k):128 * (m + k) + 128],
                                rhs=ctxT[:, p, n_sl],
                                start=(p == 0), stop=(p == 1))
                        if k == 0:
                            nc.scalar.copy(osb[:, 0, :], po[:])
                        else:
                            nc.vector.tensor_copy(osb[:, 1, :], po[:])
                nc.sync.dma_start(outT_p[:, m:m + 2, n_sl], osb[:])
            return fill
        return [make(m) for m in range(0, NO, 2)]

    # Chunk finishes are emitted one chunk late (the pair-1 finish rides as
    # the first filler of the next pair-0 body); out-proj j+1 and the
    # pair-1 QKV projections run as fillers inside attention bodies.
    qt_copies = emit_qk02_stream()
    emit_v(mid=qt_copies)
    # A13's own k blocks ride inside it just-in-time: score(i) only needs
    # kt block i//4, so blocks 1-3 land as fillers 2+ iterations ahead
    f03 = emit_attn_body(0, 3, fillers=qk_fillers(1) + qk_fillers(3, ns=(0,)))
    f13 = emit_attn_body(1, 3,
                         fillers=qk_fillers(3, ns=(1, 2, 3), split_copy=True),
                         fill_at=[0, 3, 7])
    f03()
    for j in (2, 1, 0):
        op = outproj_fillers(j + 1)
        if j > 0:
            fa = emit_attn_body(0, j, fillers=[f13] + op[:2])
            fb = emit_attn_body(1, j, fillers=op[2:])
        else:
            # out-proj(1) units start at i=1 (after mul(1,1) lands) and the
            # last one pads the PE queue between the final ctx matmuls and
            # the finish chain so the rs drain doesn't stall the PE
            fa = emit_attn_body(0, j, fillers=[f13])
            fb = emit_attn_body(1, j, fillers=op, fill_at=[1, 2, 3, 99])
        fa()
        f13 = fb
    f13()
    for fl in outproj_fillers(0, tail=True):
        fl()
    ctx.close()


def _get_nc():
    global _NC_CACHE
    if _NC_CACHE is None:
        _NC_CACHE = _build_core_kernel()
    return _NC_CACHE


def _build_masks():
    # the 128x128 causal staircase, twice side by side, so the kernel can
    # mask both heads' diagonal tiles with a single [128, 2, 128] multiply
    p = np.arange(128)[:, None]
    f = np.arange(128)[None, :]
    tri = (p <= f).astype(BF16)
    return np.concatenate([tri, tri], axis=1)


def _shard_inputs(x, Wq, Wk, Wv, Wo):
    xb = x.astype(BF16)
    masks = _build_masks()
    in_maps = []
    for c in range(N_CORES):
        b, g = divmod(c, 4)
        cols = slice(DHC * g, DHC * g + DHC)
        w_all = np.ascontiguousarray(np.concatenate(
            [Wq[:, cols], Wk[:, cols], Wv[:, cols]], axis=1).astype(BF16))
        wo_s = np.ascontiguousarray(Wo[cols, :].astype(BF16))
        xT = np.ascontiguousarray(xb[b].T)
        in_maps.append({"xT": xT, "w_all": w_all, "wo": wo_s, "masks": masks})
    return in_maps


def _unshard(results, bo):
    out = np.empty((2, S, D), np.float32)
    for b in range(2):
        acc = results[4 * b]["outT"].astype(np.float32)
        for g in range(1, 4):
            acc += results[4 * b + g]["outT"].astype(np.float32)
        out[b] = acc.T + bo.astype(np.float32)
    return out


def run(x, Wq, Wk, Wv, Wo, bo, trace=False, **spmd_kwargs):
    nc = _get_nc()
    in_maps = _shard_inputs(x, Wq, Wk, Wv, Wo)
    res = bass_utils.run_bass_kernel_spmd(
        nc, in_maps, core_ids=list(range(N_CORES)), trace=trace,
        **spmd_kwargs)
    return _unshard(res.results, bo), res


def kernel(x, Wq, Wk, Wv, Wo, bo):
    out, _ = run(np.asarray(x), np.asarray(Wq), np.asarray(Wk),
                 np.asarray(Wv), np.asarray(Wo), np.asarray(bo))
    return out


# revision 62
# speedup vs baseline: 1.0037x; 1.0037x over previous
"""Multi-head causal attention on 8 Trainium2 NeuronCores.

Sharding: core c handles batch b=c//4, head group g=c%4 (4 heads of 16).
Per-core Bass kernel computes QKV projection, causal flash-style attention
(transposed-scores layout), and the out-projection partial; the host sums
the 4 per-batch partials (the out_proj all-reduce) and adds the bias.

Layout notes (per core, S=2048 tokens, D=1024, 4 heads x dh=64):
  - xT [D, S] bf16 arrives pre-transposed from host (d_in on partitions).
  - qT/kT [128, pair, S]: partitions = head-dim; pair p holds heads 2p
    (partitions 0:64) and 2p+1 (64:128); k^T zero-padded per head
    (ktpA=[kA|0], ktpB=[0|kB]) so score matmuls are full-array K=128.
  - scoresT tile [128 k-tokens, 512 q-tokens]; exp on the scalar engine
    (the only ACT table ever loaded), exp row-sums accumulate on DVE,
    diagonal causal masks as a single GpSimd multiply per k-chunk.
  - v zero-padded per head parity so ctx matmuls are full-array (M=128)
    and the two heads accumulate additively in one PSUM bank.
  - PE pipelining: ctx matmuls for k-chunk i are emitted after the score
    matmuls of chunk i+1, so the PE never waits on the exp; independent
    "filler" matmul work (pair-1 QKV projections, deferred out-proj
    blocks) is interleaved into the attention chunks so the PE stays
    busy while scalar/vector tails drain — a mostly-idle PE window
    re-throttles the PE HAM clock gate from 2.4 to 1.2 GHz.
  - Softmax denominators: one col-tiled ones-matmul pair per chunk, DVE
    reciprocal_approx_fast, then a fused psum->sbuf normalize multiply;
    each chunk's finish is emitted after the NEXT chunk's body.
  - out^T partial [D, S] f32 is DMA'd straight from PSUM (no staging).
"""

import sys

sys.path.insert(0, "/opt/trn_rl_repo")

import numpy as np
import ml_dtypes

import concourse.bass as bass
import concourse.tile as tile
from concourse import bacc, mybir
from concourse import bass_utils

BF16 = ml_dtypes.bfloat16
F32 = mybir.dt.float32
BF = mybir.dt.bfloat16

N_CORES = 8
S = 2048          # tokens
D = 1024          # model dim
DHC = 256         # head dims per core (4 heads x 64)
DH = 64
NQ = 4            # q chunks of 512
NK = 16           # k chunks of 128
NO = 8            # d_in / d_out chunks of 128

_NC_CACHE = None


def _build_core_kernel():
    nc = bacc.Bacc("TRN2", target_bir_lowering=False, debug=False,
                   num_devices=N_CORES)
    xT = nc.dram_tensor("xT", [D, S], BF, kind="ExternalInput").ap()
    w_all = nc.dram_tensor("w_all", [D, 3 * DHC], BF, kind="ExternalInput").ap()
    wo = nc.dram_tensor("wo", [DHC, D], BF, kind="ExternalInput").ap()
    masks = nc.dram_tensor("masks", [128, 256], BF, kind="ExternalInput").ap()
    outT = nc.dram_tensor("outT", [D, S], BF, kind="ExternalOutput").ap()

    with tile.TileContext(nc) as tc:
        _emit(tc, xT, w_all, wo, masks, outT)
    nc.compile()
    return nc


def _emit(tc, xT, w_all, wo, masks, outT):
    nc = tc.nc
    EXPF = mybir.ActivationFunctionType.Exp

    from contextlib import ExitStack
    ctx = ExitStack()
    const = ctx.enter_context(tc.tile_pool(name="const", bufs=1))
    work = ctx.enter_context(tc.tile_pool(name="work", bufs=6))
    outp = ctx.enter_context(tc.tile_pool(name="outp", bufs=6))
    nrm = ctx.enter_context(tc.tile_pool(name="nrm", bufs=2))
    ps_mm = ctx.enter_context(tc.tile_pool(name="ps_mm", bufs=2, space="PSUM"))
    ps_s = ctx.enter_context(tc.tile_pool(name="ps_s", bufs=2, space="PSUM"))
    ps_c = ctx.enter_context(tc.tile_pool(name="ps_c", bufs=2, space="PSUM"))

    # ---- persistent SBUF tensors ----
    xt = const.tile([128, NO, S], BF, tag="xt")          # x^T, d_in chunks
    wa = const.tile([128, NO, 3 * DHC], BF, tag="wa")    # [Wq|Wk|Wv] slices
    wos = const.tile([128, 2, D], BF, tag="wos")         # Wo row chunks
    msk = const.tile([128, 2, 128], BF, tag="msk")       # causal staircase x2
    qt = const.tile([128, 2, S], BF, tag="qt")           # q^T per pair
    ktpA = const.tile([128, 2, S], BF, tag="ktpA")
    ktpB = const.tile([128, 2, S], BF, tag="ktpB")
    vsb = const.tile([128, NK, 4 * 128], BF, tag="vsb")
    ctxT = const.tile([128, 2, S], BF, tag="ctxT")       # normalized ctx^T
    ones = const.tile([128, DH], BF, tag="ones")

    # per-chunk DMAs, wa/xt interleaved, so compute starts on chunk 0
    # without waiting for the full weight load
    wao = w_all.rearrange("(o p) f -> p o f", p=128)
    xTo = xT.rearrange("(o p) s -> o p s", p=128)
    for o in range(NO):
        nc.sync.dma_start(wa[:, o, :], wao[:, o, :])
        nc.sync.dma_start(xt[:, o, :], xTo[o])
    nc.sync.dma_start(msk[:], masks.rearrange("p (g f) -> p g f", g=2))
    nc.sync.dma_start(wos[:], wo.rearrange("(c p) f -> p c f", p=128))
    nc.vector.memset(ones[:], 1.0)
    nc.vector.memset(ktpA[:], 0.0)
    nc.gpsimd.memset(ktpB[:], 0.0)
    nc.gpsimd.memset(vsb[:], 0.0)
    msk2 = msk[:]  # both heads' triangular mask in one [128, 2, 128] op

    # ---- QKV projections ----
    def emit_qk02_stream():
        # q and k for pair 0 together, o-loop outermost: 8 matmuls per
        # arriving x^T/w chunk keep the PE saturated (and the HAM clock
        # busy) while the input DMA streams in.  q accumulates in the two
        # ps_s tiles; k's four 512-col accumulators borrow ps_mm + ps_c.
        pq0 = ps_s.tile([128, 1024], F32, tag="ps")
        pq1 = ps_s.tile([128, 1024], F32, tag="ps")
        pk0 = ps_mm.tile([128, 512], F32, tag="mm")
        pk1 = ps_c.tile([128, 512], F32, tag="pc")
        pk2 = ps_mm.tile([128, 512], F32, tag="mm")
        pk3 = ps_c.tile([128, 512], F32, tag="pc")
        pks = [pk0, pk1, pk2, pk3]
        for o in range(NO):
            for n in range(NQ):
                # k before q: k's accumulators gate emit_v (via ps_mm), so
                # their final stop — and psum-freeing copies — land earlier
                n_sl = slice(512 * n, 512 * n + 512)
                nc.tensor.matmul(
                    pks[n][:], lhsT=wa[:, o, 256:384], rhs=xt[:, o, n_sl],
                    start=(o == 0), stop=(o == NO - 1),
                    skip_group_check=True)
                nc.tensor.matmul(
                    [pq0, pq1][n // 2][:, 512 * (n % 2):512 * (n % 2) + 512],
                    lhsT=wa[:, o, 0:128], rhs=xt[:, o, n_sl],
                    start=(o == 0), stop=(o == NO - 1),
                    skip_group_check=True)
        for n in range(NQ):
            n_sl = slice(512 * n, 512 * n + 512)
            nc.vector.tensor_copy(ktpA[0:64, 0, n_sl], pks[n][0:64, :])
            nc.scalar.copy(ktpB[64:128, 0, n_sl], pks[n][64:128, :])

        def qt_copies():  # deferred past emit_v: ps_s isn't needed until
            for g in range(2):  # attention, so don't stall the PE here
                g_sl = slice(1024 * g, 1024 * g + 1024)
                nc.vector.tensor_copy(qt[:, 0, g_sl], [pq0, pq1][g][:])
        return qt_copies

    def qk_fillers(m, ns=tuple(range(NQ)), split_copy=False):
        # pair-1 projections as independent filler units (one 512-token
        # block each: 8 accumulating matmuls + a psum->sbuf copy on the
        # scalar engine, which has slack inside attention windows);
        # split_copy spreads the two k copies across vector+scalar for
        # units that ride inside an exp-paced window
        pp = m % 2

        def make(n):
            def fill():
                n_sl = slice(512 * n, 512 * n + 512)
                pq = ps_mm.tile([128, 512], F32, tag="mm")
                for o in range(NO):
                    nc.tensor.matmul(
                        pq[:], lhsT=wa[:, o, 128 * m:128 * m + 128],
                        rhs=xt[:, o, n_sl],
                        start=(o == 0), stop=(o == NO - 1))
                if m < 2:
                    nc.scalar.copy(qt[:, pp, n_sl], pq[:])
                elif split_copy:
                    nc.vector.tensor_copy(ktpA[0:64, pp, n_sl], pq[0:64, :])
                    nc.scalar.copy(ktpB[64:128, pp, n_sl], pq[64:128, :])
                else:
                    nc.scalar.copy(ktpA[0:64, pp, n_sl], pq[0:64, :])
                    nc.scalar.copy(ktpB[64:128, pp, n_sl], pq[64:128, :])
            return fill
        return [make(n) for n in ns]

    def emit_v(mid=None):
        # v [tokens, 4*dh] = x @ Wv  (x^T chunks are the stationary side)
        for t in range(NK):
            if t == 2 and mid is not None:
                mid()  # deferred copies drain while the PE streams v
            pv = ps_mm.tile([128, 512], F32, tag="mm")
            for o in range(NO):
                nc.tensor.matmul(
                    pv[:, :DHC], lhsT=xt[:, o, 128 * t:128 * t + 128],
                    rhs=wa[:, o, 2 * DHC:3 * DHC],
                    start=(o == 0), stop=(o == NO - 1))
            pv4 = pv[:, :DHC].rearrange("p (h c) -> p h c", c=DH)
            dst4 = vsb.rearrange("p t (h c) -> p t h c", c=128)
            # even head slots hold [v|0], odd hold [0|v].  All copies on
            # DVE: queueing half on the scalar FIFO delays the first
            # attention exps behind ~7us of copies (strict FIFO order)
            nc.vector.tensor_copy(dst4[:, t, 0::2, 0:64], pv4[:, 0::2, :])
            nc.vector.tensor_copy(dst4[:, t, 1::2, 64:128], pv4[:, 1::2, :])

    # ---- attention for one (pair, q-chunk); returns deferred finisher ----
    def emit_attn_body(p, j, fillers=(), fill_at=None):
        h0 = 2 * p
        n_i = 4 * j + 4
        q_sl = slice(512 * j, 512 * j + 512)
        pc = ps_c.tile([128, 512], F32, tag="pc")
        rs = work.tile([128, 1024], BF, tag="rs")  # per-head exp row-sums
        fillers = list(fillers)
        if fill_at is None:
            spacing = max(1, n_i // (len(fillers) + 1)) if fillers else 0
            fill_at = [(k + 1) * spacing - 1 for k in range(len(fillers))]
        nfill = 0

        def emit_ctx(eT, q0, i):
            for h in (0, 1):  # head within pair; full-array M=128 matmuls
                sl = slice(512 * h + q0, 512 * h + 512)
                hl = h0 + h
                nc.tensor.matmul(
                    pc[:, q0:512],
                    lhsT=vsb[:, i, 128 * hl:128 * hl + 128],
                    rhs=eT[:, sl],
                    start=(i == 0 and h == 0), stop=(i == n_i - 1 and h == 1),
                    skip_group_check=True)

        pend = []
        for i in range(n_i):
            k_sl = slice(128 * i, 128 * i + 128)
            d = i - 4 * j
            # diagonal tiles: k-chunk i only reaches q >= 128*d in this
            # q-window; restrict all work to the valid column range.
            q0 = 128 * d if d > 0 else 0
            qv_sl = slice(512 * j + q0, 512 * j + 512)
            pss = ps_s.tile([128, 1024], F32, tag="ps")
            nc.tensor.matmul(pss[:, q0:512],
                             lhsT=ktpA[:, p, k_sl], rhs=qt[:, p, qv_sl],
                             start=True, stop=True)
            nc.tensor.matmul(pss[:, 512 + q0:1024],
                             lhsT=ktpB[:, p, k_sl], rhs=qt[:, p, qv_sl],
                             start=True, stop=True)
            # ctx matmuls run two iterations behind their scores so the PE
            # never waits on the exp -> mask chain of the same k-chunk
            if len(pend) == 2:
                emit_ctx(*pend.pop(0))
            if fillers and nfill < len(fillers) and i >= fill_at[nfill]:
                fillers[nfill]()
                nfill += 1
            eT = work.tile([128, 1024], BF, tag="exp")
            if q0:
                ev = eT.rearrange("p (g f) -> p g f", g=2)[:, :, q0:512]
                pv_ = pss.rearrange("p (g f) -> p g f", g=2)[:, :, q0:512]
                nc.scalar.activation(ev, pv_, EXPF, scale=0.125)
            else:
                nc.scalar.activation(eT[:], pss[:], EXPF, scale=0.125)
            if d >= 0:  # both heads' triangular diagonal mask in one op
                e2 = eT.rearrange("p (g f) -> p g f", g=2)[:, :, q0:q0 + 128]
                nc.vector.tensor_mul(e2, e2, msk2)
            # exp row-sum accumulation (both heads, one DVE op per k-chunk)
            if i == 0:
                nc.vector.tensor_copy(rs[:], eT[:])
            elif q0:
                rv = rs.rearrange("p (g f) -> p g f", g=2)[:, :, q0:512]
                ev2 = eT.rearrange("p (g f) -> p g f", g=2)[:, :, q0:512]
                nc.vector.tensor_add(rv, rv, ev2)
            else:
                nc.vector.tensor_add(rs[:], rs[:], eT[:])
            pend.append((eT, q0, i))
        for pe in pend:
            emit_ctx(*pe)
        for k in range(nfill, len(fillers)):
            fillers[k]()

        def finish():
            # denominators: one col-tiled ones-matmul pair on the summed
            # rows, then 1/den on DVE fused into the psum->sbuf normalize
            pd = ps_mm.tile([128, 512], F32, tag="mm")
            nc.tensor.matmul(pd[0:64, :], lhsT=ones[:], rhs=rs[:, 0:512],
                             start=True, stop=True)
            nc.tensor.matmul(pd[64:128, :], lhsT=ones[:], rhs=rs[:, 512:1024],
                             start=True, stop=True)
            rec = nrm.tile([128, 512], F32, tag="rec")
            nc.vector.reciprocal_approx_fast(rec[:], pd[:])
            nc.vector.tensor_mul(ctxT[:, p, q_sl], pc[:], rec[:])
        return finish

    # ---- out projection: outT[:, n] += wo.T @ ctxT ----
    # bf16 staging, two 128-row blocks per DMA to halve sync-queue issues
    outT_p = outT.rearrange("(mm p) s -> p mm s", p=128)

    def outproj_fillers(n, tail=False):
        n_sl = slice(512 * n, 512 * n + 512)

        def make(m):  # one filler = out rows 128m .. 128(m+2)
            def fill():
                osb = outp.tile([128, 2, 512], BF, tag="osb")
                if tail:
                    # attention psum is free at the tail: use a 2-bank tile
                    # and drain both halves with parallel scalar+DVE copies
                    po2 = ps_s.tile([128, 1024], F32, tag="ps")
                    for k in (0, 1):
                        for p in (0, 1):
                            nc.tensor.matmul(
                                po2[:, 512 * k:512 * k + 512],
                                lhsT=wos[:, p, 128 * (m + k):128 * (m + k) + 128],
                                rhs=ctxT[:, p, n_sl],
                                start=(p == 0), stop=(p == 1),
                                skip_group_check=True)
                    nc.scalar.copy(osb[:, 0, :], po2[:, 0:512])
                    nc.vector.tensor_copy(osb[:, 1, :], po2[:, 512:1024])
                else:
                    for k in (0, 1):
                        po = ps_mm.tile([128, 512], F32, tag="mm")
                        for p in (0, 1):
                            nc.tensor.matmul(
                                po[:], lhsT=wos[:, p, 128 * (m + k):128 * (m + k) + 128],
                                rhs=ctxT[:, p, n_sl],
                                start=(p == 0), stop=(p == 1))
                        if k == 0:
                            nc.scalar.copy(osb[:, 0, :], po[:])
                        else:
                            nc.vector.tensor_copy(osb[:, 1, :], po[:])
                nc.sync.dma_start(outT_p[:, m:m + 2, n_sl], osb[:])
            return fill
        return [make(m) for m in range(0, NO, 2)]

    # Chunk finishes are emitted one chunk late (the pair-1 finish rides as
    # the first filler of the next pair-0 body); out-proj j+1 and the
    # pair-1 QKV projections run as fillers inside attention bodies.
    qt_copies = emit_qk02_stream()
    emit_v(mid=qt_copies)
    # A13's own k blocks ride inside it just-in-time: score(i) only needs
    # kt block i//4, so blocks 1-3 land as fillers 2+ iterations ahead
    f03 = emit_attn_body(0, 3, fillers=qk_fillers(1) + qk_fillers(3, ns=(0,)))
    f13 = emit_attn_body(1, 3,
                         fillers=qk_fillers(3, ns=(1, 2, 3), split_copy=True),
                         fill_at=[0, 3, 7])
    f03()
    for j in (2, 1, 0):
        op = outproj_fillers(j + 1)
        if j > 0:
            fa = emit_attn_body(0, j, fillers=[f13] + op[:2])
            fb = emit_attn_body(1, j, fillers=op[2:])
        else:
            # out-proj(1) units start at i=1 (after mul(1,1) lands) and the
            # last one pads the PE queue between the final ctx matmuls and
            # the finish chain so the rs drain doesn't stall the PE
            fa = emit_attn_body(0, j, fillers=[f13])
            fb = emit_attn_body(1, j, fillers=op, fill_at=[1, 2, 3, 99])
        fa()
        f13 = fb
    f13()
    for fl in outproj_fillers(0, tail=True):
        fl()
    ctx.close()


def _get_nc():
    global _NC_CACHE
    if _NC_CACHE is None:
        _NC_CACHE = _build_core_kernel()
    return _NC_CACHE


def _build_masks():
    # the 128x128 causal staircase, twice side by side, so the kernel can
    # mask both heads' diagonal tiles with a single [128, 2, 128] multiply
    p = np.arange(128)[:, None]
    f = np.arange(128)[None, :]
    tri = (p <= f).astype(BF16)
    return np.concatenate([tri, tri], axis=1)


def _shard_inputs(x, Wq, Wk, Wv, Wo):
    xb = x.astype(BF16)
    masks = _build_masks()
    in_maps = []
    for c in range(N_CORES):
        b, g = divmod(c, 4)
        cols = slice(DHC * g, DHC * g + DHC)
        w_all = np.ascontiguousarray(np.concatenate(
            [Wq[:, cols], Wk[:, cols], Wv[:, cols]], axis=1).astype(BF16))
        wo_s = np.ascontiguousarray(Wo[cols, :].astype(BF16))
        xT = np.ascontiguousarray(xb[b].T)
        in_maps.append({"xT": xT, "w_all": w_all, "wo": wo_s, "masks": masks})
    return in_maps


def _unshard(results, bo):
    out = np.empty((2, S, D), np.float32)
    for b in range(2):
        acc = results[4 * b]["outT"].astype(np.float32)
        for g in range(1, 4):
            acc += results[4 * b + g]["outT"].astype(np.float32)
        out[b] = acc.T + bo.astype(np.float32)
    return out


def run(x, Wq, Wk, Wv, Wo, bo, trace=False, **spmd_kwargs):
    nc = _get_nc()
    in_maps = _shard_inputs(x, Wq, Wk, Wv, Wo)
    res = bass_utils.run_bass_kernel_spmd(
        nc, in_maps, core_ids=list(range(N_CORES)), trace=trace,
        **spmd_kwargs)
    return _unshard(res.results, bo), res


def kernel(x, Wq, Wk, Wv, Wo, bo):
    out, _ = run(np.asarray(x), np.asarray(Wq), np.asarray(Wk),
                 np.asarray(Wv), np.asarray(Wo), np.asarray(bo))
    return out


# revision 64
# speedup vs baseline: 1.0213x; 1.0176x over previous
"""Multi-head causal attention on 8 Trainium2 NeuronCores.

Sharding: core c handles batch b=c//4, head group g=c%4 (4 heads of 16).
Per-core Bass kernel computes QKV projection, causal flash-style attention
(transposed-scores layout), and the out-projection partial; the host sums
the 4 per-batch partials (the out_proj all-reduce) and adds the bias.

Layout notes (per core, S=2048 tokens, D=1024, 4 heads x dh=64):
  - xT [D, S] bf16 arrives pre-transposed from host (d_in on partitions).
  - qT/kT [128, pair, S]: partitions = head-dim; pair p holds heads 2p
    (partitions 0:64) and 2p+1 (64:128); k^T zero-padded per head
    (ktpA=[kA|0], ktpB=[0|kB]) so score matmuls are full-array K=128.
  - scoresT tile [128 k-tokens, 512 q-tokens]; exp on the scalar engine
    (the only ACT table ever loaded), exp row-sums accumulate on DVE,
    diagonal causal masks as a single GpSimd multiply per k-chunk.
  - v zero-padded per head parity so ctx matmuls are full-array (M=128)
    and the two heads accumulate additively in one PSUM bank.
  - PE pipelining: ctx matmuls for k-chunk i are emitted after the score
    matmuls of chunk i+1, so the PE never waits on the exp; independent
    "filler" matmul work (pair-1 QKV projections, deferred out-proj
    blocks) is interleaved into the attention chunks so the PE stays
    busy while scalar/vector tails drain — a mostly-idle PE window
    re-throttles the PE HAM clock gate from 2.4 to 1.2 GHz.
  - Softmax denominators: one col-tiled ones-matmul pair per chunk, DVE
    reciprocal_approx_fast, then a fused psum->sbuf normalize multiply;
    each chunk's finish is emitted after the NEXT chunk's body.
  - out^T partial [D, S] f32 is DMA'd straight from PSUM (no staging).
"""

import sys

sys.path.insert(0, "/opt/trn_rl_repo")

import numpy as np
import ml_dtypes

import concourse.bass as bass
import concourse.tile as tile
from concourse import bacc, mybir
from concourse import bass_utils

BF16 = ml_dtypes.bfloat16
F32 = mybir.dt.float32
BF = mybir.dt.bfloat16

N_CORES = 8
S = 2048          # tokens
D = 1024          # model dim
DHC = 256         # head dims per core (4 heads x 64)
DH = 64
NQ = 4            # q chunks of 512
NK = 16           # k chunks of 128
NO = 8            # d_in / d_out chunks of 128

_NC_CACHE = None


def _build_core_kernel():
    nc = bacc.Bacc("TRN2", target_bir_lowering=False, debug=False,
                   num_devices=N_CORES)
    xT = nc.dram_tensor("xT", [D, S], BF, kind="ExternalInput").ap()
    w_all = nc.dram_tensor("w_all", [D, 3 * DHC], BF, kind="ExternalInput").ap()
    wo = nc.dram_tensor("wo", [DHC, D], BF, kind="ExternalInput").ap()
    masks = nc.dram_tensor("masks", [128, 256], BF, kind="ExternalInput").ap()
    outT = nc.dram_tensor("outT", [D, S], BF, kind="ExternalOutput").ap()

    with tile.TileContext(nc) as tc:
        _emit(tc, xT, w_all, wo, masks, outT)
    nc.compile()
    return nc


def _emit(tc, xT, w_all, wo, masks, outT):
    nc = tc.nc
    EXPF = mybir.ActivationFunctionType.Exp

    from contextlib import ExitStack
    ctx = ExitStack()
    const = ctx.enter_context(tc.tile_pool(name="const", bufs=1))
    work = ctx.enter_context(tc.tile_pool(name="work", bufs=6))
    outp = ctx.enter_context(tc.tile_pool(name="outp", bufs=8))
    nrm = ctx.enter_context(tc.tile_pool(name="nrm", bufs=2))
    ps_mm = ctx.enter_context(tc.tile_pool(name="ps_mm", bufs=2, space="PSUM"))
    ps_s = ctx.enter_context(tc.tile_pool(name="ps_s", bufs=2, space="PSUM"))
    ps_c = ctx.enter_context(tc.tile_pool(name="ps_c", bufs=2, space="PSUM"))

    # ---- persistent SBUF tensors ----
    xt = const.tile([128, NO, S], BF, tag="xt")          # x^T, d_in chunks
    wa = const.tile([128, NO, 3 * DHC], BF, tag="wa")    # [Wq|Wk|Wv] slices
    wos = const.tile([128, 2, D], BF, tag="wos")         # Wo row chunks
    msk = const.tile([128, 2, 128], BF, tag="msk")       # causal staircase x2
    qt = const.tile([128, 2, S], BF, tag="qt")           # q^T per pair
    ktpA = const.tile([128, 2, S], BF, tag="ktpA")
    ktpB = const.tile([128, 2, S], BF, tag="ktpB")
    vsb = const.tile([128, NK, 4 * 128], BF, tag="vsb")
    ctxT = const.tile([128, 2, S], BF, tag="ctxT")       # normalized ctx^T
    ones = const.tile([128, DH], BF, tag="ones")

    # per-chunk DMAs, wa/xt interleaved, so compute starts on chunk 0
    # without waiting for the full weight load
    wao = w_all.rearrange("(o p) f -> p o f", p=128)
    xTo = xT.rearrange("(o p) s -> o p s", p=128)
    for o in range(NO):
        nc.sync.dma_start(wa[:, o, :], wao[:, o, :])
        nc.sync.dma_start(xt[:, o, :], xTo[o])
    nc.sync.dma_start(msk[:], masks.rearrange("p (g f) -> p g f", g=2))
    nc.sync.dma_start(wos[:], wo.rearrange("(c p) f -> p c f", p=128))
    nc.vector.memset(ones[:], 1.0)
    nc.vector.memset(ktpA[:], 0.0)
    nc.gpsimd.memset(ktpB[:], 0.0)
    nc.gpsimd.memset(vsb[:], 0.0)
    msk2 = msk[:]  # both heads' triangular mask in one [128, 2, 128] op

    # ---- QKV projections ----
    def emit_qk02_stream():
        # q and k for pair 0 together, o-loop outermost: 8 matmuls per
        # arriving x^T/w chunk keep the PE saturated (and the HAM clock
        # busy) while the input DMA streams in.  q accumulates in the two
        # ps_s tiles; k's four 512-col accumulators borrow ps_mm + ps_c.
        pq0 = ps_s.tile([128, 1024], F32, tag="ps")
        pq1 = ps_s.tile([128, 1024], F32, tag="ps")
        pk0 = ps_mm.tile([128, 512], F32, tag="mm")
        pk1 = ps_c.tile([128, 512], F32, tag="pc")
        pk2 = ps_mm.tile([128, 512], F32, tag="mm")
        pk3 = ps_c.tile([128, 512], F32, tag="pc")
        pks = [pk0, pk1, pk2, pk3]
        for o in range(NO):
            for n in range(NQ):
                # k before q: k's accumulators gate emit_v (via ps_mm), so
                # their final stop — and psum-freeing copies — land earlier
                n_sl = slice(512 * n, 512 * n + 512)
                nc.tensor.matmul(
                    pks[n][:], lhsT=wa[:, o, 256:384], rhs=xt[:, o, n_sl],
                    start=(o == 0), stop=(o == NO - 1),
                    skip_group_check=True)
                nc.tensor.matmul(
                    [pq0, pq1][n // 2][:, 512 * (n % 2):512 * (n % 2) + 512],
                    lhsT=wa[:, o, 0:128], rhs=xt[:, o, n_sl],
                    start=(o == 0), stop=(o == NO - 1),
                    skip_group_check=True)
        for n in range(NQ):
            n_sl = slice(512 * n, 512 * n + 512)
            nc.vector.tensor_copy(ktpA[0:64, 0, n_sl], pks[n][0:64, :])
            nc.scalar.copy(ktpB[64:128, 0, n_sl], pks[n][64:128, :])

        def qt_copies():  # deferred past emit_v: ps_s isn't needed until
            for g in range(2):  # attention, so don't stall the PE here
                g_sl = slice(1024 * g, 1024 * g + 1024)
                nc.vector.tensor_copy(qt[:, 0, g_sl], [pq0, pq1][g][:])
        return qt_copies

    def qk_fillers(m, ns=tuple(range(NQ)), split_copy=False):
        # pair-1 projections as independent filler units (one 512-token
        # block each: 8 accumulating matmuls + a psum->sbuf copy on the
        # scalar engine, which has slack inside attention windows);
        # split_copy spreads the two k copies across vector+scalar for
        # units that ride inside an exp-paced window
        pp = m % 2

        def make(n):
            def fill():
                n_sl = slice(512 * n, 512 * n + 512)
                pq = ps_mm.tile([128, 512], F32, tag="mm")
                for o in range(NO):
                    nc.tensor.matmul(
                        pq[:], lhsT=wa[:, o, 128 * m:128 * m + 128],
                        rhs=xt[:, o, n_sl],
                        start=(o == 0), stop=(o == NO - 1))
                if m < 2:
                    nc.scalar.copy(qt[:, pp, n_sl], pq[:])
                elif split_copy:
                    nc.vector.tensor_copy(ktpA[0:64, pp, n_sl], pq[0:64, :])
                    nc.scalar.copy(ktpB[64:128, pp, n_sl], pq[64:128, :])
                else:
                    nc.scalar.copy(ktpA[0:64, pp, n_sl], pq[0:64, :])
                    nc.scalar.copy(ktpB[64:128, pp, n_sl], pq[64:128, :])
            return fill
        return [make(n) for n in ns]

    def emit_v(mid=None):
        # v [tokens, 4*dh] = x @ Wv  (x^T chunks are the stationary side)
        for t in range(NK):
            if t == 2 and mid is not None:
                mid()  # deferred copies drain while the PE streams v
            pv = ps_mm.tile([128, 512], F32, tag="mm")
            for o in range(NO):
                nc.tensor.matmul(
                    pv[:, :DHC], lhsT=xt[:, o, 128 * t:128 * t + 128],
                    rhs=wa[:, o, 2 * DHC:3 * DHC],
                    start=(o == 0), stop=(o == NO - 1))
            pv4 = pv[:, :DHC].rearrange("p (h c) -> p h c", c=DH)
            dst4 = vsb.rearrange("p t (h c) -> p t h c", c=128)
            # even head slots hold [v|0], odd hold [0|v].  All copies on
            # DVE: queueing half on the scalar FIFO delays the first
            # attention exps behind ~7us of copies (strict FIFO order)
            nc.vector.tensor_copy(dst4[:, t, 0::2, 0:64], pv4[:, 0::2, :])
            nc.vector.tensor_copy(dst4[:, t, 1::2, 64:128], pv4[:, 1::2, :])

    # ---- attention for one (pair, q-chunk); returns deferred finisher ----
    def emit_attn_body(p, j, fillers=(), fill_at=None):
        h0 = 2 * p
        n_i = 4 * j + 4
        q_sl = slice(512 * j, 512 * j + 512)
        pc = ps_c.tile([128, 512], F32, tag="pc")
        rs = work.tile([128, 1024], BF, tag="rs")  # per-head exp row-sums
        fillers = list(fillers)
        if fill_at is None:
            spacing = max(1, n_i // (len(fillers) + 1)) if fillers else 0
            fill_at = [(k + 1) * spacing - 1 for k in range(len(fillers))]
        nfill = 0

        def emit_ctx(eT, q0, i):
            for h in (0, 1):  # head within pair; full-array M=128 matmuls
                sl = slice(512 * h + q0, 512 * h + 512)
                hl = h0 + h
                nc.tensor.matmul(
                    pc[:, q0:512],
                    lhsT=vsb[:, i, 128 * hl:128 * hl + 128],
                    rhs=eT[:, sl],
                    start=(i == 0 and h == 0), stop=(i == n_i - 1 and h == 1),
                    skip_group_check=True)

        pend = []
        for i in range(n_i):
            k_sl = slice(128 * i, 128 * i + 128)
            d = i - 4 * j
            # diagonal tiles: k-chunk i only reaches q >= 128*d in this
            # q-window; restrict all work to the valid column range.
            q0 = 128 * d if d > 0 else 0
            qv_sl = slice(512 * j + q0, 512 * j + 512)
            pss = ps_s.tile([128, 1024], F32, tag="ps")
            nc.tensor.matmul(pss[:, q0:512],
                             lhsT=ktpA[:, p, k_sl], rhs=qt[:, p, qv_sl],
                             start=True, stop=True)
            nc.tensor.matmul(pss[:, 512 + q0:1024],
                             lhsT=ktpB[:, p, k_sl], rhs=qt[:, p, qv_sl],
                             start=True, stop=True)
            # ctx matmuls run two iterations behind their scores so the PE
            # never waits on the exp -> mask chain of the same k-chunk
            if len(pend) == 2:
                emit_ctx(*pend.pop(0))
            if fillers and nfill < len(fillers) and i >= fill_at[nfill]:
                fillers[nfill]()
                nfill += 1
            eT = work.tile([128, 1024], BF, tag="exp")
            if q0:
                ev = eT.rearrange("p (g f) -> p g f", g=2)[:, :, q0:512]
                pv_ = pss.rearrange("p (g f) -> p g f", g=2)[:, :, q0:512]
                nc.scalar.activation(ev, pv_, EXPF, scale=0.125)
            else:
                nc.scalar.activation(eT[:], pss[:], EXPF, scale=0.125)
            if d >= 0:  # both heads' triangular diagonal mask in one op
                e2 = eT.rearrange("p (g f) -> p g f", g=2)[:, :, q0:q0 + 128]
                nc.vector.tensor_mul(e2, e2, msk2)
            # exp row-sum accumulation (both heads, one DVE op per k-chunk)
            if i == 0:
                nc.vector.tensor_copy(rs[:], eT[:])
            elif q0:
                rv = rs.rearrange("p (g f) -> p g f", g=2)[:, :, q0:512]
                ev2 = eT.rearrange("p (g f) -> p g f", g=2)[:, :, q0:512]
                nc.vector.tensor_add(rv, rv, ev2)
            else:
                nc.vector.tensor_add(rs[:], rs[:], eT[:])
            pend.append((eT, q0, i))
        for pe in pend:
            emit_ctx(*pe)
        for k in range(nfill, len(fillers)):
            fillers[k]()

        def finish():
            # denominators: one col-tiled ones-matmul pair on the summed
            # rows, then 1/den on DVE fused into the psum->sbuf normalize
            pd = ps_mm.tile([128, 512], F32, tag="mm")
            nc.tensor.matmul(pd[0:64, :], lhsT=ones[:], rhs=rs[:, 0:512],
                             start=True, stop=True)
            nc.tensor.matmul(pd[64:128, :], lhsT=ones[:], rhs=rs[:, 512:1024],
                             start=True, stop=True)
            rec = nrm.tile([128, 512], F32, tag="rec")
            nc.vector.reciprocal_approx_fast(rec[:], pd[:])
            nc.vector.tensor_mul(ctxT[:, p, q_sl], pc[:], rec[:])
        return finish

    # ---- out projection: outT[:, n] += wo.T @ ctxT ----
    # bf16 staging, two 128-row blocks per DMA to halve sync-queue issues
    outT_p = outT.rearrange("(mm p) s -> p mm s", p=128)

    def outproj_fillers(n, tail=False):
        n_sl = slice(512 * n, 512 * n + 512)

        def make(m):  # one filler = out rows 128m .. 128(m+2)
            def fill():
                osb = outp.tile([128, 2, 512], BF, tag="osb")
                # alternate tail units between ps_s and ps_mm so the 3rd
                # unit doesn't wait on the 1st unit's staging copies
                if tail and (m // 2) % 2 == 0:
                    # attention psum is free at the tail: use a 2-bank tile
                    # and drain both halves with parallel scalar+DVE copies
                    po2 = ps_s.tile([128, 1024], F32, tag="ps")
                    for k in (0, 1):
                        for p in (0, 1):
                            nc.tensor.matmul(
                                po2[:, 512 * k:512 * k + 512],
                                lhsT=wos[:, p, 128 * (m + k):128 * (m + k) + 128],
                                rhs=ctxT[:, p, n_sl],
                                start=(p == 0), stop=(p == 1),
                                skip_group_check=True)
                    nc.scalar.copy(osb[:, 0, :], po2[:, 0:512])
                    nc.vector.tensor_copy(osb[:, 1, :], po2[:, 512:1024])
                else:
                    for k in (0, 1):
                        po = ps_mm.tile([128, 512], F32, tag="mm")
                        for p in (0, 1):
                            nc.tensor.matmul(
                                po[:], lhsT=wos[:, p, 128 * (m + k):128 * (m + k) + 128],
                                rhs=ctxT[:, p, n_sl],
                                start=(p == 0), stop=(p == 1))
                        if k == 0:
                            nc.scalar.copy(osb[:, 0, :], po[:])
                        else:
                            nc.vector.tensor_copy(osb[:, 1, :], po[:])
                nc.sync.dma_start(outT_p[:, m:m + 2, n_sl], osb[:])
            return fill
        return [make(m) for m in range(0, NO, 2)]

    # Chunk finishes are emitted one chunk late (the pair-1 finish rides as
    # the first filler of the next pair-0 body); out-proj j+1 and the
    # pair-1 QKV projections run as fillers inside attention bodies.
    qt_copies = emit_qk02_stream()
    emit_v(mid=qt_copies)
    # A13's own k blocks ride inside it just-in-time: score(i) only needs
    # kt block i//4, so blocks 1-3 land as fillers 2+ iterations ahead
    f03 = emit_attn_body(0, 3, fillers=qk_fillers(1) + qk_fillers(3, ns=(0,)))
    f13 = emit_attn_body(1, 3,
                         fillers=qk_fillers(3, ns=(1, 2, 3), split_copy=True),
                         fill_at=[0, 3, 7])
    f03()
    for j in (2, 1, 0):
        op = outproj_fillers(j + 1)
        if j > 0:
            fa = emit_attn_body(0, j, fillers=[f13] + op[:2])
            fb = emit_attn_body(1, j, fillers=op[2:])
        else:
            # out-proj(1) units start at i=1 (after mul(1,1) lands) and the
            # last one pads the PE queue between the final ctx matmuls and
            # the finish chain so the rs drain doesn't stall the PE
            fa = emit_attn_body(0, j, fillers=[f13])
            fb = emit_attn_body(1, j, fillers=op, fill_at=[1, 2, 3, 99])
        fa()
        f13 = fb
    f13()
    for fl in outproj_fillers(0, tail=True):
        fl()
    ctx.close()


def _get_nc():
    global _NC_CACHE
    if _NC_CACHE is None:
        _NC_CACHE = _build_core_kernel()
    return _NC_CACHE


def _build_masks():
    # the 128x128 causal staircase, twice side by side, so the kernel can
    # mask both heads' diagonal tiles with a single [128, 2, 128] multiply
    p = np.arange(128)[:, None]
    f = np.arange(128)[None, :]
    tri = (p <= f).astype(BF16)
    return np.concatenate([tri, tri], axis=1)


def _shard_inputs(x, Wq, Wk, Wv, Wo):
    xb = x.astype(BF16)
    masks = _build_masks()
    in_maps = []
    for c in range(N_CORES):
        b, g = divmod(c, 4)
        cols = slice(DHC * g, DHC * g + DHC)
        w_all = np.ascontiguousarray(np.concatenate(
            [Wq[:, cols], Wk[:, cols], Wv[:, cols]], axis=1).astype(BF16))
        wo_s = np.ascontiguousarray(Wo[cols, :].astype(BF16))
        xT = np.ascontiguousarray(xb[b].T)
        in_maps.append({"xT": xT, "w_all": w_all, "wo": wo_s, "masks": masks})
    return in_maps


def _unshard(results, bo):
    out = np.empty((2, S, D), np.float32)
    for b in range(2):
        acc = results[4 * b]["outT"].astype(np.float32)
        for g in range(1, 4):
            acc += results[4 * b + g]["outT"].astype(np.float32)
        out[b] = acc.T + bo.astype(np.float32)
    return out


def run(x, Wq, Wk, Wv, Wo, bo, trace=False, **spmd_kwargs):
    nc = _get_nc()
    in_maps = _shard_inputs(x, Wq, Wk, Wv, Wo)
    res = bass_utils.run_bass_kernel_spmd(
        nc, in_maps, core_ids=list(range(N_CORES)), trace=trace,
        **spmd_kwargs)
    return _unshard(res.results, bo), res


def kernel(x, Wq, Wk, Wv, Wo, bo):
    out, _ = run(np.asarray(x), np.asarray(Wq), np.asarray(Wk),
                 np.asarray(Wv), np.asarray(Wo), np.asarray(bo))
    return out
